# revision 39
# baseline (speedup 1.0000x reference)
"""Causal multi-head self-attention on 8 Trainium2 NeuronCores.

Problem: x[4, 2048, 2048] fp32, w_q/w_k/w_v/w_o [2048, 2048] fp32,
16 heads x d_head=128. out = softmax(causal(QK^T/sqrt(128))) V @ w_o.

Sharding: tensor-parallel over heads. Core c owns heads {2c, 2c+1}:
computes Q^T/K^T ([d_head, tokens]) and V ([tokens, d_head]) for its
heads from the full x (streamed pre-transposed as x^T), runs per-head
causal attention producing attnT [d_head, tokens], then the partial
output o_c = attn_c @ w_o[head rows]; the host sums the 8 partials.

v2 design notes (PE-occupancy focused — TRN2 drops the PE clock from
2.4 to 1.2 GHz after any stall, so the whole schedule is built to
keep the tensor engine continuously fed):
- Probabilities ride in fp16: exp(scale*s - 8) fits comfortably
  (scores are O(+-6)); denominators/numerators are both scaled by
  e^-8 so the ratio is unchanged. fp16 halves DVE cost (2x/4x modes)
  and is full-rate on the PE.
- Scores for BOTH heads of a chunk land in one [128, 2, 512] PSUM
  pair-tile -> ONE exp activation per chunk covers both heads, so the
  Act engine has 2x latency slack vs the PE's score+PV work.
- Causal masking multiplies by constant 0/1 fp16 mask tiles on the
  DVE (fast mode) instead of gpsimd affine_select per tile.
- Softmax denominator: fp16 chunk accumulation (DVE) then one PE
  matmul pair (selector columns) gives both heads' row sums in a
  [2, 512] PSUM tile; reciprocal_approx_fast (single DVE op, ~18
  bits) replaces the 3.3us-per-call exact reciprocal; a rank-1 PE
  matmul broadcasts 1/sum across partitions; the at = ps_at * inv
  multiply drains attention PSUM.
- O-projection of block j is deferred until after block j+1's first
  two score pairs are emitted, so the PE chews fresh score work while
  the denominator chain (DVE) finishes - no per-block PE bubble.
- PSUM->SBUF output drains alternate DVE/gpsimd so neither engine
  rate-limits the O-projection matmul stream.
- PSUM budget exactly 8 banks: psS 2x[128,2,512] (4) + psAT 2x[128,
  512] (2) + psO 2x[128,512] (2); projections reuse the psS tag.
"""

import contextlib
from collections import deque

import numpy as np

import concourse.bass as bass
from concourse import bacc
import concourse.mybir as mybir
from concourse.tile import TileContext
from concourse.bass_utils import run_bass_kernel_spmd

B = 4
S = 2048
D = 2048
NH = 16
DH = 128
N_CORES = 8
HPC = NH // N_CORES          # heads per core = 2
HD = HPC * DH                # head dims per core = 256
KO = D // 128                # contraction chunks = 16
TSB = 512                    # projection token superblock
NSB = S // TSB               # 4
QSB = 512                    # attention q superblock
NQSB = S // QSB              # 4
SCALE = float(1.0 / np.sqrt(DH))
CSHIFT = -8.0                # exp(s*scale - 8): keeps fp16 probs in range

F32 = mybir.dt.float32
F32R = mybir.dt.float32r
F16 = mybir.dt.float16

_CACHED = {}


def build(loop_r: int | None = None):
    nc = bacc.Bacc("TRN2", target_bir_lowering=False, debug=False)
    xT = nc.dram_tensor("xT", [B, D, S], F32, kind="ExternalInput")
    wq = nc.dram_tensor("wq", [D, HD], F32, kind="ExternalInput")
    wk = nc.dram_tensor("wk", [D, HD], F32, kind="ExternalInput")
    wv = nc.dram_tensor("wv", [D, HD], F32, kind="ExternalInput")
    wo = nc.dram_tensor("wo", [HD, D], F32, kind="ExternalInput")
    out = nc.dram_tensor("out", [B, S, D], F32, kind="ExternalOutput")

    env = {
        "xT_v": xT.ap().bitcast(F32R),
        "out": out,
    }

    with TileContext(nc) as tc:
        with tc.tile_pool(name="const", bufs=1) as const, \
             tc.tile_pool(name="big", bufs=1) as big, \
             tc.tile_pool(name="stream", bufs=2) as stream, \
             tc.tile_pool(name="work", bufs=4) as work, \
             tc.tile_pool(name="ps", bufs=2, space="PSUM") as ps:

            # ---- weights / constants (resident) ----
            wq_sb = const.tile([128, KO, HD], F32R)
            wk_sb = const.tile([128, KO, HD], F32R)
            wv_sb = const.tile([128, KO, HD], F32R)
            wo_sb = const.tile([128, HPC, D], F32R)

            env.update(wq_sb=wq_sb, wk_sb=wk_sb, wv_sb=wv_sb, wo_sb=wo_sb,
                       big=big, stream=stream, work=work, ps=ps)

            # Batch-0 superblock-0 xt is DMA'd interleaved with wq so the
            # first Q matmul chain starts ~2MB in, not after all weights.
            xt0 = stream.tile([128, KO, TSB], F32R, tag="xt", bufs=2)
            wq_v = wq.ap().rearrange("(ko p) m -> p ko m", p=128).bitcast(F32R)
            wk_v = wk.ap().rearrange("(ko p) m -> p ko m", p=128).bitcast(F32R)
            wv_v = wv.ap().rearrange("(ko p) m -> p ko m", p=128).bitcast(F32R)
            xT_v = env["xT_v"]
            for ko in range(KO):
                nc.sync.dma_start(wq_sb[:, ko], wq_v[:, ko])
                nc.sync.dma_start(xt0[:, ko], xT_v[0, ko * 128:(ko + 1) * 128,
                                                  0:TSB])
            for ko in range(KO):
                nc.sync.dma_start(wk_sb[:, ko], wk_v[:, ko])
            for ko in range(KO):
                nc.sync.dma_start(wv_sb[:, ko], wv_v[:, ko])
            nc.sync.dma_start(
                wo_sb, wo.ap().rearrange("(c p) n -> p c n", p=128).bitcast(F32R))
            env["xt0"] = xt0

            # per-partition bias column for exp(s*scale + CSHIFT)
            cbias = const.tile([128, 1], F32)
            nc.vector.memset(cbias, CSHIFT)
            env["cbias"] = cbias

            # all-ones [128,128] fp16: ones_mat^T @ acc_h gives the softmax
            # denominator already broadcast across every output partition
            tmpm = const.tile([128, 128], F32)
            ones_mat = const.tile([128, 128], F16)
            nc.vector.memset(tmpm, 1.0)
            nc.vector.tensor_copy(ones_mat, tmpm)

            # causal masks (0/1), both heads' halves identical. For a
            # diagonal chunk at d = c*128 - j*512, column g of the q-block
            # is kept iff g >= d + p. One mask tile per d in {0,..,384}.
            mtmp = const.tile([128, 512], F32)
            masks = []
            for d in range(0, 512, 128):
                mk = const.tile([128, 2, 512], F16, name=f"mask{d}")
                nc.vector.memset(mtmp, 1.0)
                nc.gpsimd.affine_select(
                    out=mtmp, in_=mtmp, compare_op=mybir.AluOpType.is_ge,
                    fill=0.0, base=-d, pattern=[[1, 512]],
                    channel_multiplier=-1)
                nc.vector.tensor_copy(mk[:, 0], mtmp)
                nc.vector.tensor_copy(mk[:, 1], mtmp)
                masks.append(mk)

            env.update(ones_mat=ones_mat, masks=masks)

            loop_cm = (tc.For_i(0, loop_r, 1) if loop_r else
                       contextlib.nullcontext())
            with loop_cm:
                _batches(nc, env)

    nc.compile()
    return nc


def _proj(nc, env, b, pending_oproj=None):
    """Q^T/K^T/V projections for batch b into qt/kt/v SBUF tiles.

    pending_oproj = (prev_b, at_tiles): the previous batch's last
    O-projection, emitted after superblock 0's Q matmul chain so ~7us of
    independent PE work covers its denominator-chain latency.
    """
    stream, work, ps = env["stream"], env["work"], env["ps"]
    wq_sb, wk_sb, wv_sb = env["wq_sb"], env["wk_sb"], env["wv_sb"]
    xT_v = env["xT_v"]
    qt_sb, kt_sb, v_sb = env["qt_sb"], env["kt_sb"], env["v_sb"]
    opend = (_o_proj_groups(nc, env, pending_oproj[0], NQSB - 1,
                            pending_oproj[1])
             if pending_oproj is not None else deque())

    for s in range(NSB):
        if b == 0 and s == 0:
            xt = env["xt0"]          # pre-loaded interleaved with wq
        else:
            xt = stream.tile([128, KO, TSB], F32R, tag="xt", bufs=2)
            for ko in range(KO):
                nc.sync.dma_start(
                    xt[:, ko],
                    xT_v[b, ko * 128:(ko + 1) * 128, s * TSB:(s + 1) * TSB])
        # Q^T then K^T: [dh, tokens] = W_chunk^T @ x^T; both 128-rows of
        # head-dim go in one [128, 2, 512] psum pair-tile.
        for (wsb, dst) in ((wq_sb, qt_sb), (wk_sb, kt_sb)):
            psq = ps.tile([128, 2, TSB], F32, tag="psS", bufs=2)
            for m in range(HPC):
                for ko in range(KO):
                    nc.tensor.matmul(
                        psq[:, m], wsb[:, ko, m * 128:(m + 1) * 128], xt[:, ko],
                        start=(ko == 0), stop=(ko == KO - 1))
            nc.scalar.copy(dst[:, :, s * TSB:(s + 1) * TSB], psq)
            if opend:
                opend.popleft()()
        # V: [tokens, dh] = x @ W_v, two 128-token chunks per pair-tile,
        # downcast to fp16 on drain.
        for tp in range(TSB // 256):
            psv = ps.tile([128, 2, TSB], F32, tag="psS", bufs=2)
            for ti in range(2):
                t = tp * 2 + ti
                for ko in range(KO):
                    nc.tensor.matmul(
                        psv[:, ti, 0:HD],
                        xt[:, ko, t * 128:(t + 1) * 128], wv_sb[:, ko],
                        start=(ko == 0), stop=(ko == KO - 1))
            tc0 = s * (TSB // 128) + tp * 2
            nc.scalar.copy(v_sb[:, tc0:tc0 + 2, :], psv[:, :, 0:HD])
            if opend:
                opend.popleft()()


def _o_proj_groups(nc, env, b, j, at_tiles):
    """Emit-closures for the 16 output-projection groups of q-superblock
    j: callers interleave them into other PE work so the 2-slot psO
    rotation never outruns its DVE/Act drains."""
    work, ps = env["work"], env["ps"]
    wo_sb, out = env["wo_sb"], env["out"]

    def mk(t, n):
        def emit():
            ps_o = ps.tile([128, 512], F32, tag="psO", bufs=2, name="ps_o")
            for h in range(HPC):
                nc.tensor.matmul(
                    ps_o, at_tiles[h][:, t * 128:(t + 1) * 128],
                    wo_sb[:, h, n * 512:(n + 1) * 512],
                    start=(h == 0), stop=(h == HPC - 1))
            o_st = work.tile([128, 512], F32, tag="ost", bufs=4, name="o_st")
            if (t * 4 + n) % 2 == 0:
                nc.vector.tensor_copy(o_st, ps_o)
            else:
                nc.scalar.copy(o_st, ps_o)
            nc.sync.dma_start(
                out.ap()[b, j * QSB + t * 128:j * QSB + (t + 1) * 128,
                         n * 512:(n + 1) * 512], o_st)
        return emit

    return deque(mk(t, n) for t in range(QSB // 128) for n in range(D // 512))


def _o_proj(nc, env, b, j, at_tiles):
    """Fused output projection for q-superblock j of batch b (burst)."""
    for emit in _o_proj_groups(nc, env, b, j, at_tiles):
        emit()


def _attn(nc, env, b):
    """Causal attention + deferred fused O-projection for batch b."""
    work, ps = env["work"], env["ps"]
    qt_sb, kt_sb, v_sb = env["qt_sb"], env["kt_sb"], env["v_sb"]
    ones_mat = env["ones_mat"]
    masks = env["masks"]

    at_prev = None
    for j in range(NQSB):
        nkc = 4 * (j + 1)

        def score_pair(c):
            # causal narrowing: on diagonal chunks only q-cols >= c*128
            # matter; keep matmul width >= 256 for f32r full rate.
            if c >= 4 * j:
                off = min(c * 128 - j * QSB, QSB - 256)
            else:
                off = 0
            pss = ps.tile([128, 2, QSB], F32, tag="psS", bufs=2)
            for h in range(HPC):
                nc.tensor.matmul(
                    pss[:, h, off:], kt_sb[:, h, c * 128:(c + 1) * 128],
                    qt_sb[:, h, j * QSB + off:(j + 1) * QSB],
                    start=True, stop=True)
            return pss, off

        # three score pairs in flight before the deferred O-projection:
        # ~2.5us of PE work covering the previous block's denominator
        # chain (sum-broadcast -> recip -> at-mul).
        npre = min(3, nkc)
        pend = deque()
        for c in range(npre):
            pend.append(score_pair(c))
        # previous block's O-projection: interleaved 2 groups per chunk
        # iteration below, so its psO drains ride the chunk loop's slack
        opend = (_o_proj_groups(nc, env, b, j - 1, at_prev)
                 if at_prev is not None else deque())

        acc = work.tile([128, 2, QSB], F16, tag="acc", bufs=2)
        ps_at = [ps.tile([128, QSB], F32, tag="psAT", bufs=2, name=f"psat{h}")
                 for h in range(HPC)]
        pt_last, off_last = None, 0
        for c in range(nkc):
            pss, off = pend.popleft()
            pt = work.tile([128, 2, QSB], F16, tag="pt", bufs=3)
            nc.scalar.activation(
                pt[:, :, off:], pss[:, :, off:],
                mybir.ActivationFunctionType.Exp, bias=env["cbias"],
                scale=SCALE)
            if c >= 4 * j:
                # causal: multiply by the 0/1 mask for this diagonal chunk
                msk = masks[(c * 128 - j * QSB) // 128]
                nc.vector.tensor_mul(
                    pt[:, :, off:], pt[:, :, off:], msk[:, :, off:])
            # the last chunk skips the DVE accumulate: its contribution
            # rides the sum-broadcast matmul directly (keeps the PE-side
            # denominator chain free of DVE latency)
            if c == 0:
                nc.vector.tensor_copy(acc, pt)
            elif c < nkc - 1:
                nc.vector.tensor_add(
                    acc[:, :, off:], acc[:, :, off:], pt[:, :, off:])
            else:
                pt_last, off_last = pt, off
            for h in range(HPC):
                nc.tensor.matmul(
                    ps_at[h][:, off:], v_sb[:, c, h * 128:(h + 1) * 128],
                    pt[:, h, off:],
                    start=(c == 0), stop=(c == nkc - 1))
            if c + npre < nkc:
                pend.append(score_pair(c + npre))
            for _ in range(2):
                if opend:
                    opend.popleft()()
        while opend:
            opend.popleft()()

        # per-head softmax denominator, broadcast across partitions by a
        # ones-matrix matmul pair (acc + last chunk's pt); reciprocal'd
        # into SBUF; at = ps_at * (1/denom) drains the attention psum.
        at_tiles = []
        for h in range(HPC):
            ps_bc = ps.tile([128, QSB], F32, tag="psO", bufs=2,
                            name=f"psbc{h}")
            nc.tensor.matmul(ps_bc, ones_mat, acc[:, h],
                             start=True, stop=False)
            nc.tensor.matmul(ps_bc[:, off_last:], ones_mat,
                             pt_last[:, h, off_last:],
                             start=False, stop=True)
            inv_bc = work.tile([128, QSB], F32, tag="invbc", bufs=2,
                               name=f"invbc{h}")
            with nc.allow_low_precision(
                    reason="~18-bit reciprocal: plenty for 1e-2 tolerance"):
                nc.vector.reciprocal_approx_fast(inv_bc, ps_bc)
            at = work.tile([128, QSB], F32R, tag="at", bufs=2,
                           name=f"at{h}")
            nc.vector.tensor_mul(at, ps_at[h], inv_bc)
            at_tiles.append(at)
        at_prev = at_tiles
    return at_prev


def _batches(nc, env):
    big = env["big"]
    # qt/kt/v are written by batch b+1's projections only after batch b's
    # attention has fully consumed them; the PE runs batches in order, so
    # single-buffered residents are safe and save SBUF.
    env["qt_sb"] = big.tile([128, HPC, S], F32R, tag="qt", bufs=1, name="qt")
    env["kt_sb"] = big.tile([128, HPC, S], F32R, tag="kt", bufs=1, name="kt")
    env["v_sb"] = big.tile([128, S // 128, HD], F16, tag="v", bufs=1, name="v")
    pending = None
    for b in range(B):
        _proj(nc, env, b, pending_oproj=pending)
        at_last = _attn(nc, env, b)
        pending = (b, at_last)
    _o_proj(nc, env, B - 1, NQSB - 1, at_last)


def kernel(x, w_q, w_k, w_v, w_o, _trace=False):
    x = np.ascontiguousarray(np.asarray(x, dtype=np.float32))
    xT = np.ascontiguousarray(x.transpose(0, 2, 1))
    in_maps = []
    for c in range(N_CORES):
        sl = slice(c * HD, (c + 1) * HD)
        in_maps.append({
            "xT": xT,
            "wq": np.ascontiguousarray(np.asarray(w_q, np.float32)[:, sl]),
            "wk": np.ascontiguousarray(np.asarray(w_k, np.float32)[:, sl]),
            "wv": np.ascontiguousarray(np.asarray(w_v, np.float32)[:, sl]),
            "wo": np.ascontiguousarray(np.asarray(w_o, np.float32)[sl, :]),
        })
    if "nc" not in _CACHED:
        _CACHED["nc"] = build()
    res = run_bass_kernel_spmd(
        _CACHED["nc"], in_maps, core_ids=list(range(N_CORES)),
        trace=_trace)
    if _trace:
        _CACHED["last_result"] = res
    acc = np.zeros((B, S, D), dtype=np.float64)
    for r in res.results:
        acc += r["out"]
    return acc.astype(np.float32)
